# revision 1
# baseline (speedup 1.0000x reference)
"""ConvLSTM cell kernel for Trainium2 (8 NeuronCores, data-parallel over batch).

Strategy (per core, one batch element):
- Conv(x;wx) + Conv(h;wh) computed as one fused 53-channel 3x3 conv via
  shifted-window matmuls on a column-padded flat layout (width 258).
  SBUF buffer T1 [106, UA] holds channels [x;h] twice: A = rows at strip
  offset, B = A shifted one image row. Per 512-px output window, 6 matmuls
  accumulate into PSUM [128, 512]:
    3 "pair" MMs  K=106 (dy=-1 via A, dy=0 via B), dx in {-1,0,1}
    3 "single" MMs K=53 (dy=+1 via A),             dx in {-1,0,1}
  Matmul inputs are bf16 (1 cyc/row on PE); accumulation fp32 in PSUM.
- Gates: one ACT sigmoid per window (tanh done as 2*sigmoid(2x)-1 so the
  ACT LUT stays warm and every op runs 128 partitions wide); cg-gate weights
  and bias pre-scaled by 2 on the host.
- Gate-type partitions are regrouped into window-quadrant layout with 4
  SBUF->SBUF DMAs per window so all elementwise ops run at nch=128.
- Elementwise state update in fp32 on DVE; c input and cc/ch outputs are
  bf16 in DRAM (halves state DMA traffic; ~2e-3 added rounding error).
- Dispatch: direct bass2jax shard_map executor with donated output buffers
  zero-filled on-device, so only real payloads cross the host link.

Host side packs inputs: padded flat bf16 image [53, L], fp32->bf16 weights
in lhsT layout (cg block pre-scaled), padded bf16 c, and strips the column
padding from the outputs.
"""
import sys
from contextlib import ExitStack

import numpy as np
import ml_dtypes

sys.path.insert(0, "/opt/trn_rl_repo")

import concourse.bass as bass  # noqa: E402
import concourse.tile as tile  # noqa: E402
from concourse import bacc, mybir  # noqa: E402
from concourse.bass_utils import run_bass_kernel_spmd  # noqa: E402

BF16 = mybir.dt.bfloat16
F32 = mybir.dt.float32
AF = mybir.ActivationFunctionType
ALU = mybir.AluOpType

# problem constants (hardcoded per spec)
B = 8
CX, CH = 21, 32
C = CX + CH           # 53
CO = 128
H = W = 256
P = 258               # padded width
HS = 32               # output rows per strip
NSTRIP = H // HS      # 8
UA = 1 + (HS + 2) * P + 1      # 8774 elements per T1 partition
L = 1 + 259 * P + 1            # 66824 flat padded length (+guards)
FLAT = H * P                   # 66048 padded output positions
NW = 16               # full 512-px windows per strip
NTAIL = FLAT // NSTRIP - NW * 512   # 64
GRPS = NW // 4        # 4 groups of 4 windows

_CACHED_NC = None


def _build_nc():
    nc = bacc.Bacc("TRN2", target_bir_lowering=False, debug=False, num_devices=B)

    xh = nc.dram_tensor("xh", [C, L], BF16, kind="ExternalInput").ap()
    cpad = nc.dram_tensor("cpad", [CH, FLAT], BF16, kind="ExternalInput").ap()
    wpair = nc.dram_tensor("wpair", [2 * C, 3 * CO], BF16, kind="ExternalInput").ap()
    wsing = nc.dram_tensor("wsing", [C, 3 * CO], BF16, kind="ExternalInput").ap()
    bvec = nc.dram_tensor("bvec", [CO, 1], F32, kind="ExternalInput").ap()
    occ = nc.dram_tensor("occ", [CH, FLAT], BF16, kind="ExternalOutput").ap()
    och = nc.dram_tensor("och", [CH, FLAT], BF16, kind="ExternalOutput").ap()

    with tile.TileContext(nc) as tc, ExitStack() as ctx:
        wpool = ctx.enter_context(tc.tile_pool(name="w", bufs=1))
        t1pool = ctx.enter_context(tc.tile_pool(name="t1", bufs=2))
        pspool = ctx.enter_context(tc.tile_pool(name="ps", bufs=4, space="PSUM"))
        pstail = ctx.enter_context(tc.tile_pool(name="pst", bufs=2, space="PSUM"))
        gpool = ctx.enter_context(tc.tile_pool(name="g", bufs=6))
        spool = ctx.enter_context(tc.tile_pool(name="stk", bufs=2))
        cpool = ctx.enter_context(tc.tile_pool(name="cb", bufs=2))
        epool = ctx.enter_context(tc.tile_pool(name="ew", bufs=2))
        tpool = ctx.enter_context(tc.tile_pool(name="tl", bufs=2))

        wp = wpool.tile([2 * C, 3 * CO], BF16)
        nc.sync.dma_start(wp[:], wpair[:])
        ws = wpool.tile([C, 3 * CO], BF16)
        nc.sync.dma_start(ws[:], wsing[:])
        bias = wpool.tile([CO, 1], F32)
        nc.sync.dma_start(bias[:], bvec[:])

        def conv_window(t1, j, n, pool):
            """6 accumulating matmuls -> PSUM tile [128, n] for window j."""
            pg = pool.tile([CO, n], F32)
            for dxi in range(3):
                F = j * 512 + dxi
                nc.tensor.matmul(pg[:], wp[:, dxi * CO:(dxi + 1) * CO],
                                 t1[0:2 * C, F:F + n],
                                 start=(dxi == 0), stop=False)
            for dxi in range(3):
                F = j * 512 + 2 * P + dxi
                nc.tensor.matmul(pg[:], ws[:, dxi * CO:(dxi + 1) * CO],
                                 t1[0:C, F:F + n],
                                 start=False, stop=(dxi == 2))
            return pg

        for s in range(NSTRIP):
            y0 = HS * s
            t1 = t1pool.tile([2 * C, UA], BF16)
            nc.sync.dma_start(t1[0:C, :], xh[:, y0 * P: y0 * P + UA])
            nc.scalar.dma_start(t1[C:2 * C, :], xh[:, (y0 + 1) * P: (y0 + 1) * P + UA])
            base = y0 * P

            for grp in range(GRPS):
                goff = base + grp * 2048
                cbuf = cpool.tile([CO, 512], BF16)
                for q in range(4):
                    eng = nc.sync if q % 2 == 0 else nc.scalar
                    eng.dma_start(cbuf[q * CH:(q + 1) * CH, :],
                                  cpad[:, goff + q * 512: goff + (q + 1) * 512])

                stk = spool.tile([CO, 2048], F32)
                for q in range(4):
                    j = grp * 4 + q
                    pg = conv_window(t1, j, 512, pspool)
                    gates = gpool.tile([CO, 512], F32)
                    nc.scalar.activation(gates[:], pg[:], AF.Sigmoid, bias=bias[:])
                    for g in range(4):
                        eng = nc.sync if (q + g) % 2 == 0 else nc.scalar
                        eng.dma_start(stk[q * CH:(q + 1) * CH, g * 512:(g + 1) * 512],
                                      gates[g * CH:(g + 1) * CH, :])

                Fg = stk[:, 0:512]
                Ig = stk[:, 512:1024]
                CGg = stk[:, 1024:1536]
                Og = stk[:, 1536:2048]
                # cg = 2*sigmoid(2g)-1  (weights for cg block pre-scaled x2)
                nc.vector.tensor_scalar(CGg, CGg, 2.0, -1.0, ALU.mult, ALU.add)
                t1s = epool.tile([CO, 512], F32)
                nc.vector.tensor_tensor(t1s[:], Fg, cbuf[:], ALU.mult)
                t2s = epool.tile([CO, 512], F32)
                nc.vector.tensor_tensor(t2s[:], Ig, CGg, ALU.mult)
                ccb = epool.tile([CO, 512], BF16)
                nc.vector.tensor_tensor(ccb[:], t1s[:], t2s[:], ALU.add)
                tcs = epool.tile([CO, 512], F32)
                nc.scalar.activation(tcs[:], ccb[:], AF.Sigmoid, scale=2.0)
                nc.vector.tensor_scalar(tcs[:], tcs[:], 2.0, -1.0, ALU.mult, ALU.add)
                chb = epool.tile([CO, 512], BF16)
                nc.vector.tensor_tensor(chb[:], Og, tcs[:], ALU.mult)

                for q in range(4):
                    eng = nc.sync if q % 2 == 0 else nc.scalar
                    eng.dma_start(occ[:, goff + q * 512: goff + (q + 1) * 512],
                                  ccb[q * CH:(q + 1) * CH, :])
                    eng = nc.scalar if q % 2 == 0 else nc.sync
                    eng.dma_start(och[:, goff + q * 512: goff + (q + 1) * 512],
                                  chb[q * CH:(q + 1) * CH, :])

            # tail window j=16 (64 px), nch=32 path
            n = NTAIL
            toff = base + NW * 512
            pg = conv_window(t1, NW, n, pstail)
            gt = tpool.tile([CO, n], F32)
            nc.scalar.activation(gt[:], pg[:], AF.Sigmoid, bias=bias[:])
            ct = tpool.tile([CH, n], BF16)
            nc.sync.dma_start(ct[:], cpad[:, toff: toff + n])
            # regroup gate quadrants to base partition 0 (walrus requires equal
            # base partitions for both SBUF inputs of tensor_tensor)
            stkt = tpool.tile([CH, 4 * n], F32)
            for g in range(4):
                eng = nc.sync if g % 2 == 0 else nc.scalar
                eng.dma_start(stkt[:, g * n:(g + 1) * n], gt[g * CH:(g + 1) * CH, :])
            nc.vector.tensor_scalar(stkt[:, 2 * n:3 * n], stkt[:, 2 * n:3 * n],
                                    2.0, -1.0, ALU.mult, ALU.add)
            t1t = tpool.tile([CH, n], F32)
            nc.vector.tensor_tensor(t1t[:], stkt[:, 0:n], ct[:], ALU.mult)
            t2t = tpool.tile([CH, n], F32)
            nc.vector.tensor_tensor(t2t[:], stkt[:, n:2 * n], stkt[:, 2 * n:3 * n],
                                    ALU.mult)
            cct = tpool.tile([CH, n], BF16)
            nc.vector.tensor_tensor(cct[:], t1t[:], t2t[:], ALU.add)
            tct = tpool.tile([CH, n], F32)
            nc.scalar.activation(tct[:], cct[:], AF.Sigmoid, scale=2.0)
            nc.vector.tensor_scalar(tct[:], tct[:], 2.0, -1.0, ALU.mult, ALU.add)
            cht = tpool.tile([CH, n], BF16)
            nc.vector.tensor_tensor(cht[:], stkt[:, 3 * n:4 * n], tct[:], ALU.mult)
            nc.sync.dma_start(occ[:, toff: toff + n], cct[:])
            nc.scalar.dma_start(och[:, toff: toff + n], cht[:])

    nc.compile()
    return nc


def _pack_xh(x_b, h_b, flat=None):
    """[21,256,256] + [32,256,256] fp32 -> [53, L] bf16 flat padded."""
    if flat is None:
        flat = np.zeros((C, L), dtype=ml_dtypes.bfloat16)
    body = flat[:, 1:1 + 259 * P].reshape(C, 259, P)
    body[0:CX, 1:257, 1:257] = x_b.astype(ml_dtypes.bfloat16)
    body[CX:C, 1:257, 1:257] = h_b.astype(ml_dtypes.bfloat16)
    return flat


def _pack_w(wx, wh, bx):
    wfull = np.concatenate([wx, wh], axis=1).astype(np.float32)  # [128,53,3,3]
    wfull = wfull.copy()
    wfull[2 * CH:3 * CH] *= 2.0          # cg gate: tanh via 2*sigmoid(2x)-1
    wpair = np.zeros((2 * C, 3, CO), np.float32)
    wsing = np.zeros((C, 3, CO), np.float32)
    for dxi in range(3):
        wpair[0:C, dxi, :] = wfull[:, :, 0, dxi].T
        wpair[C:2 * C, dxi, :] = wfull[:, :, 1, dxi].T
        wsing[:, dxi, :] = wfull[:, :, 2, dxi].T
    bvec = bx.astype(np.float32).copy()
    bvec[2 * CH:3 * CH] *= 2.0
    return (wpair.reshape(2 * C, 3 * CO).astype(ml_dtypes.bfloat16),
            wsing.reshape(C, 3 * CO).astype(ml_dtypes.bfloat16),
            bvec.reshape(CO, 1))


def _pack_c(c_b, out=None):
    """[32,256,256] fp32 -> [32, FLAT] bf16 padded-width layout."""
    if out is None:
        out = np.zeros((CH, H, P), ml_dtypes.bfloat16)
    else:
        out = out.reshape(CH, H, P)
    out[:, :, 1:257] = c_b.astype(ml_dtypes.bfloat16)
    return out.reshape(CH, FLAT)


_RUNNER = None


def _make_runner(nc):
    """Sharded PJRT executor mirroring run_bass_via_pjrt, but with the
    donated zero output buffers created on-device (saves their host->device
    transfer over the axon tunnel)."""
    import jax
    from jax.sharding import Mesh, PartitionSpec, NamedSharding
    from jax.experimental.shard_map import shard_map
    from concourse.bass2jax import (_bass_exec_p, install_neuronx_cc_hook,
                                    partition_id_tensor)
    import jax.numpy as jnp

    install_neuronx_cc_hook()
    partition_name = nc.partition_id_tensor.name if nc.partition_id_tensor else None
    in_names, out_names, out_avals = [], [], []
    for alloc in nc.m.functions[0].allocations:
        if not isinstance(alloc, mybir.MemoryLocationSet):
            continue
        name = alloc.memorylocations[0].name
        if alloc.kind == "ExternalInput":
            if name != partition_name:
                in_names.append(name)
        elif alloc.kind == "ExternalOutput":
            out_names.append(name)
            out_avals.append(jax.core.ShapedArray(tuple(alloc.tensor_shape),
                                                  mybir.dt.np(alloc.dtype)))
    n_params = len(in_names)
    all_in = list(in_names) + list(out_names)
    if partition_name is not None:
        all_in.append(partition_name)

    def _body(*args):
        operands = list(args)
        if partition_name is not None:
            operands.append(partition_id_tensor())
        return tuple(_bass_exec_p.bind(
            *operands, out_avals=tuple(out_avals), in_names=tuple(all_in),
            out_names=tuple(out_names), lowering_input_output_aliases=(),
            sim_require_finite=True, sim_require_nnan=True, nc=nc))

    devices = jax.devices()[:B]
    mesh = Mesh(np.asarray(devices), ("core",))
    n_outs = len(out_names)
    fn = jax.jit(
        shard_map(_body, mesh=mesh,
                  in_specs=(PartitionSpec("core"),) * (n_params + n_outs),
                  out_specs=(PartitionSpec("core"),) * n_outs,
                  check_rep=False),
        donate_argnums=tuple(range(n_params, n_params + n_outs)),
        keep_unused=True)
    sh = NamedSharding(mesh, PartitionSpec("core"))
    zshapes = tuple((B * a.shape[0], *a.shape[1:]) for a in out_avals)
    zdtypes = tuple(a.dtype for a in out_avals)
    zeros_fn = jax.jit(
        lambda: tuple(jnp.zeros(s, d) for s, d in zip(zshapes, zdtypes)),
        out_shardings=(sh,) * n_outs)

    global _runner_state
    _runner_state = {"fn": fn, "zeros_fn": zeros_fn, "sh": sh,
                     "in_names": in_names, "out_names": out_names,
                     "out_avals": out_avals}

    def run(concat_map):
        # concat_map values are already stacked (B*dim0, ...) host arrays
        concat_in = [jax.device_put(concat_map[nm], sh) for nm in in_names]
        outs = fn(*concat_in, *zeros_fn())
        return {name: np.asarray(outs[i]).reshape(B, *out_avals[i].shape)
                for i, name in enumerate(out_names)}

    return run


def kernel(x, h, c, wx, bx, wh):
    global _CACHED_NC, _RUNNER
    x, h, c = np.asarray(x), np.asarray(h), np.asarray(c)
    wx, bx, wh = np.asarray(wx), np.asarray(bx), np.asarray(wh)
    if _CACHED_NC is None:
        _CACHED_NC = _build_nc()
        _RUNNER = _make_runner(_CACHED_NC)

    wpair, wsing, bvec = _pack_w(wx, wh, bx)
    xh_cat = np.zeros((B * C, L), ml_dtypes.bfloat16)
    c_cat = np.zeros((B * CH, FLAT), ml_dtypes.bfloat16)
    for b in range(B):
        _pack_xh(x[b], h[b], flat=xh_cat[b * C:(b + 1) * C])
        _pack_c(c[b], out=c_cat[b * CH:(b + 1) * CH])
    concat_map = {
        "xh": xh_cat,
        "cpad": c_cat,
        "wpair": np.tile(wpair, (B, 1)),
        "wsing": np.tile(wsing, (B, 1)),
        "bvec": np.tile(bvec, (B, 1)),
    }

    res = _RUNNER(concat_map)
    ch_out = res["och"].reshape(B, CH, H, P)[:, :, :, 1:257].astype(np.float32)
    cc_out = res["occ"].reshape(B, CH, H, P)[:, :, :, 1:257].astype(np.float32)
    return (ch_out, cc_out)



# revision 7
# speedup vs baseline: 2.7749x; 2.7749x over previous
"""ConvLSTM cell kernel for Trainium2 (8 NeuronCores, data-parallel over batch).

Strategy (per core, one batch element):
- Conv(x;wx) + Conv(h;wh) computed as one fused 53-channel 3x3 conv via
  shifted-window matmuls on a column-padded flat layout (width 258).
  The whole padded image lives in SBUF as T1 [106, LX] bf16: partitions
  0:53 hold the flat image, partitions 53:106 the same shifted one image
  row. 132 uniform 512-px windows (33 groups of 4) cover the padded
  image plus a little zero tail -- no strip loop, no tail special case.
  Per window, 6 matmuls accumulate into PSUM [128, 512]:
    3 "pair" MMs  K=106 (dy=-1 via A, dy=0 via B), dx in {-1,0,1}
    3 "single" MMs K=53 (dy=+1 via A),             dx in {-1,0,1}
- Gates: one ACT sigmoid per window writes fp16 into a [128, 2048] group
  tile (tanh done as 2*sigmoid(2x)-1 so the ACT LUT stays warm); cg-gate
  weights pre-scaled by 2 on the host.
- Gate-type partitions are regrouped into window-quadrant layout with 4
  multi-dim SBUF->SBUF DMAs per group (partition-block <-> column-block
  patterns) so all elementwise ops run at nch=128. fp16 gates halve the
  regroup traffic vs fp32 and round 8x finer than bf16.
- Elementwise state update on DVE; c input and cc/ch outputs are fp16 in
  DRAM (same traffic as bf16, 4x less rounding error).
- Dispatch: direct bass2jax shard_map executor, one execution per call
  (the kernel writes every output element so no zero-init/donation
  round), compiled via fast_dispatch_compile for C++ fast-path dispatch.

Host side packs inputs: padded flat bf16 image [53, LXB], fp32->bf16
weights in lhsT layout (cg block pre-scaled), fp16 padded c, and strips
the column padding from the outputs.
"""
import sys
from contextlib import ExitStack

import numpy as np
import ml_dtypes

sys.path.insert(0, "/opt/trn_rl_repo")

import concourse.bass as bass  # noqa: E402
import concourse.tile as tile  # noqa: E402
from concourse import bacc, mybir  # noqa: E402

BF16 = mybir.dt.bfloat16
FP16 = mybir.dt.float16
F32 = mybir.dt.float32
AF = mybir.ActivationFunctionType
ALU = mybir.AluOpType

# problem constants (hardcoded per spec)
B = 8
CX, CH = 21, 32
C = CX + CH           # 53
CO = 128
H = W = 256
P = 258               # padded width
NWIN = 132            # uniform 512-px windows (covers image + zero tail)
GRPS = NWIN // 4      # 33 groups of 4 windows
FLATX = NWIN * 512    # 67584 output positions (66048 real + zero tail)
FLAT = H * P          # 66048 real padded output positions
LX = NWIN * 512 + 2 * P + 4    # 68104 t1 columns (max window read + margin)
LXB = LX + P                   # 68362 xh DRAM length (B copy reads +P)
CHUNK = 12 * 512               # t1 load chunk (12 windows of columns)

_CACHED_NC = None


def _build_nc():
    nc = bacc.Bacc("TRN2", target_bir_lowering=False, debug=False, num_devices=B)

    xh = nc.dram_tensor("xh", [C, LXB], BF16, kind="ExternalInput").ap()
    cpad = nc.dram_tensor("cpad", [CH, FLATX], FP16, kind="ExternalInput").ap()
    wpair = nc.dram_tensor("wpair", [2 * C, 3 * CO], BF16, kind="ExternalInput").ap()
    wsing = nc.dram_tensor("wsing", [C, 3 * CO], BF16, kind="ExternalInput").ap()
    bvec = nc.dram_tensor("bvec", [CO, 1], F32, kind="ExternalInput").ap()
    occ = nc.dram_tensor("occ", [CH, FLATX], FP16, kind="ExternalOutput").ap()
    och = nc.dram_tensor("och", [CH, FLATX], FP16, kind="ExternalOutput").ap()

    with tile.TileContext(nc) as tc, ExitStack() as ctx:
        wpool = ctx.enter_context(tc.tile_pool(name="w", bufs=1))
        t1pool = ctx.enter_context(tc.tile_pool(name="t1", bufs=1))
        pspool = ctx.enter_context(tc.tile_pool(name="ps", bufs=6, space="PSUM"))
        gpool = ctx.enter_context(tc.tile_pool(name="g", bufs=2))
        spool = ctx.enter_context(tc.tile_pool(name="stk", bufs=2))
        cpool = ctx.enter_context(tc.tile_pool(name="cb", bufs=2))
        epool = ctx.enter_context(tc.tile_pool(name="ew", bufs=2))

        wp = wpool.tile([2 * C, 3 * CO], BF16)
        nc.scalar.dma_start(wp[:], wpair[:])
        ws = wpool.tile([C, 3 * CO], BF16)
        nc.scalar.dma_start(ws[:], wsing[:])
        bias = wpool.tile([CO, 1], F32)
        nc.scalar.dma_start(bias[:], bvec[:])

        # whole padded image in SBUF, loaded in column chunks so early
        # windows can start before the full load completes
        t1 = t1pool.tile([2 * C, LX], BF16)
        for k0 in range(0, LX, CHUNK):
            k1 = min(k0 + CHUNK, LX)
            nc.sync.dma_start(t1[0:C, k0:k1], xh[:, k0:k1])
            nc.gpsimd.dma_start(t1[C:2 * C, k0:k1], xh[:, P + k0:P + k1])

        for grp in range(GRPS):
            goff = grp * 2048
            cbuf = cpool.tile([CO, 512], FP16)
            # SBUF side is plain 2D [128, 512] with partition p = c*4 + q
            # (channel-major interleave); the 3D pattern lives on the DRAM
            # side whose stream order (c, q, n) matches p-major order.
            nc.scalar.dma_start(
                cbuf[:],
                cpad[:, goff:goff + 2048].rearrange("c (q n) -> c q n", q=4))

            gbig = gpool.tile([CO, 2048], FP16)
            for q in range(4):
                j = grp * 4 + q
                pg = pspool.tile([CO, 512], F32)
                for dxi in range(3):
                    F = j * 512 + dxi
                    nc.tensor.matmul(pg[:], wp[:, dxi * CO:(dxi + 1) * CO],
                                     t1[0:2 * C, F:F + 512],
                                     start=(dxi == 0), stop=False)
                for dxi in range(3):
                    F = j * 512 + 2 * P + dxi
                    nc.tensor.matmul(pg[:], ws[:, dxi * CO:(dxi + 1) * CO],
                                     t1[0:C, F:F + 512],
                                     start=False, stop=(dxi == 2))
                nc.scalar.activation(gbig[:, q * 512:(q + 1) * 512], pg[:],
                                     AF.Sigmoid, bias=bias[:])

            # regroup gate-type partitions -> interleaved window-quadrant
            # layout (p = c*4 + q), one multi-dim SBUF->SBUF DMA per gate:
            # src has its partition dim (c) outermost, dst is plain 2D
            stk = spool.tile([CO, 2048], FP16)
            qeng = (nc.sync, nc.gpsimd, nc.sync, nc.gpsimd)
            for g in range(4):
                qeng[g].dma_start(
                    stk[:, g * 512:(g + 1) * 512],
                    gbig[g * CH:(g + 1) * CH, :].rearrange("c (q n) -> c q n", q=4))

            Fg = stk[:, 0:512]
            Ig = stk[:, 512:1024]
            CGg = stk[:, 1024:1536]
            Og = stk[:, 1536:2048]
            # cg = 2*sigmoid(2g)-1  (weights for cg block pre-scaled x2)
            nc.vector.tensor_scalar(CGg, CGg, 2.0, -1.0, ALU.mult, ALU.add)
            t1s = epool.tile([CO, 512], F32)
            nc.vector.tensor_tensor(t1s[:], Fg, cbuf[:], ALU.mult)
            t2s = epool.tile([CO, 512], F32)
            nc.vector.tensor_tensor(t2s[:], Ig, CGg, ALU.mult)
            ccb = epool.tile([CO, 512], FP16)
            nc.vector.tensor_tensor(ccb[:], t1s[:], t2s[:], ALU.add)
            tcs = epool.tile([CO, 512], F32)
            nc.scalar.activation(tcs[:], ccb[:], AF.Sigmoid, scale=2.0)
            nc.vector.tensor_scalar(tcs[:], tcs[:], 2.0, -1.0, ALU.mult, ALU.add)
            chb = epool.tile([CO, 512], FP16)
            nc.vector.tensor_tensor(chb[:], Og, tcs[:], ALU.mult)

            nc.sync.dma_start(
                occ[:, goff:goff + 2048].rearrange("c (q n) -> c q n", q=4),
                ccb[:])
            nc.gpsimd.dma_start(
                och[:, goff:goff + 2048].rearrange("c (q n) -> c q n", q=4),
                chb[:])

    nc.compile()
    return nc


def _pack_xh(x_b, h_b, flat=None):
    """[21,256,256] + [32,256,256] fp32 -> [53, LXB] bf16 flat padded."""
    if flat is None:
        flat = np.zeros((C, LXB), dtype=ml_dtypes.bfloat16)
    body = flat[:, 1:1 + 259 * P].reshape(C, 259, P)
    body[0:CX, 1:257, 1:257] = x_b.astype(ml_dtypes.bfloat16)
    body[CX:C, 1:257, 1:257] = h_b.astype(ml_dtypes.bfloat16)
    return flat


def _pack_w(wx, wh, bx):
    wfull = np.concatenate([wx, wh], axis=1).astype(np.float32)  # [128,53,3,3]
    wfull = wfull.copy()
    wfull[2 * CH:3 * CH] *= 2.0          # cg gate: tanh via 2*sigmoid(2x)-1
    wpair = np.zeros((2 * C, 3, CO), np.float32)
    wsing = np.zeros((C, 3, CO), np.float32)
    for dxi in range(3):
        wpair[0:C, dxi, :] = wfull[:, :, 0, dxi].T
        wpair[C:2 * C, dxi, :] = wfull[:, :, 1, dxi].T
        wsing[:, dxi, :] = wfull[:, :, 2, dxi].T
    bvec = bx.astype(np.float32).copy()
    bvec[2 * CH:3 * CH] *= 2.0
    return (wpair.reshape(2 * C, 3 * CO).astype(ml_dtypes.bfloat16),
            wsing.reshape(C, 3 * CO).astype(ml_dtypes.bfloat16),
            bvec.reshape(CO, 1))


def _pack_c(c_b, out=None):
    """[32,256,256] fp32 -> [32, FLATX] fp16 padded-width layout."""
    if out is None:
        out = np.zeros((CH, FLATX), np.float16)
    body = out[:, :FLAT].reshape(CH, H, P)
    body[:, :, 1:257] = c_b.astype(np.float16)
    return out


_RUNNER = None


def _make_runner(nc):
    """Sharded PJRT executor: single execution per call (the kernel writes
    every output element, so no zero-init/donation round is needed) compiled
    with bass_effect suppressed for C++ fast-path dispatch."""
    import jax
    from jax.sharding import Mesh, PartitionSpec, NamedSharding
    from jax.experimental.shard_map import shard_map
    from concourse.bass2jax import (_bass_exec_p, install_neuronx_cc_hook,
                                    partition_id_tensor, fast_dispatch_compile)

    install_neuronx_cc_hook()
    partition_name = nc.partition_id_tensor.name if nc.partition_id_tensor else None
    in_names, out_names, out_avals = [], [], []
    for alloc in nc.m.functions[0].allocations:
        if not isinstance(alloc, mybir.MemoryLocationSet):
            continue
        name = alloc.memorylocations[0].name
        if alloc.kind == "ExternalInput":
            if name != partition_name:
                in_names.append(name)
        elif alloc.kind == "ExternalOutput":
            out_names.append(name)
            out_avals.append(jax.core.ShapedArray(tuple(alloc.tensor_shape),
                                                  mybir.dt.np(alloc.dtype)))
    all_in = list(in_names)
    if partition_name is not None:
        all_in.append(partition_name)

    def _body(*args):
        operands = list(args)
        if partition_name is not None:
            operands.append(partition_id_tensor())
        return tuple(_bass_exec_p.bind(
            *operands, out_avals=tuple(out_avals), in_names=tuple(all_in),
            out_names=tuple(out_names), lowering_input_output_aliases=(),
            sim_require_finite=True, sim_require_nnan=True, nc=nc))

    devices = jax.devices()[:B]
    mesh = Mesh(np.asarray(devices), ("core",))
    sh = NamedSharding(mesh, PartitionSpec("core"))
    in_structs = []
    for alloc in nc.m.functions[0].allocations:
        if not isinstance(alloc, mybir.MemoryLocationSet):
            continue
        name = alloc.memorylocations[0].name
        if alloc.kind == "ExternalInput" and name != partition_name:
            in_structs.append(jax.ShapeDtypeStruct(
                (B * alloc.tensor_shape[0], *alloc.tensor_shape[1:]),
                mybir.dt.np(alloc.dtype), sharding=sh))
    fn = fast_dispatch_compile(
        lambda: jax.jit(
            shard_map(_body, mesh=mesh,
                      in_specs=(PartitionSpec("core"),) * len(in_names),
                      out_specs=(PartitionSpec("core"),) * len(out_names),
                      check_rep=False),
            keep_unused=True).lower(*in_structs).compile())

    global _runner_state
    _runner_state = {"fn": fn, "sh": sh,
                     "in_names": in_names, "out_names": out_names,
                     "out_avals": out_avals}

    def run(concat_map):
        # concat_map values are already stacked (B*dim0, ...) host arrays
        concat_in = [jax.device_put(concat_map[nm], sh) for nm in in_names]
        outs = fn(*concat_in)
        return {name: np.asarray(outs[i]).reshape(B, *out_avals[i].shape)
                for i, name in enumerate(out_names)}

    return run


def kernel(x, h, c, wx, bx, wh):
    global _CACHED_NC, _RUNNER
    x, h, c = np.asarray(x), np.asarray(h), np.asarray(c)
    wx, bx, wh = np.asarray(wx), np.asarray(bx), np.asarray(wh)
    if _CACHED_NC is None:
        _CACHED_NC = _build_nc()
        _RUNNER = _make_runner(_CACHED_NC)

    wpair, wsing, bvec = _pack_w(wx, wh, bx)
    xh_cat = np.zeros((B * C, LXB), ml_dtypes.bfloat16)
    c_cat = np.zeros((B * CH, FLATX), np.float16)
    for b in range(B):
        _pack_xh(x[b], h[b], flat=xh_cat[b * C:(b + 1) * C])
        _pack_c(c[b], out=c_cat[b * CH:(b + 1) * CH])
    concat_map = {
        "xh": xh_cat,
        "cpad": c_cat,
        "wpair": np.tile(wpair, (B, 1)),
        "wsing": np.tile(wsing, (B, 1)),
        "bvec": np.tile(bvec, (B, 1)),
    }

    res = _RUNNER(concat_map)
    ch_out = res["och"][:, :, :FLAT].reshape(B, CH, H, P)[:, :, :, 1:257].astype(np.float32)
    cc_out = res["occ"][:, :, :FLAT].reshape(B, CH, H, P)[:, :, :, 1:257].astype(np.float32)
    return (ch_out, cc_out)
